# revision 1
# baseline (speedup 1.0000x reference)
"""Trainium2 Bass kernel for nn_ExpandingLinear.

Computation (see reference):
    x_exp = concat([x, x[:, p0] * v0, x_exp1[:, p1] * v1], axis=1)   # [B, 2176]
    W     = scatter_add(weight_vals at [weight_rows, weight_cols])    # [2048, 2176]
    b     = scatter_add(bias_vals at bias_idx)                        # [2048]
    out   = x_exp @ W.T + b                                           # [B, 2048]

Sharding: data-parallel over the batch dim across 8 NeuronCores (1024 rows
per core); the weight/bias/embed parameters are replicated.

Host-side prep is limited to sharding and parameter/layout preparation
(batch split, x transpose, COO->dense weight densification, embed parent-chain
resolution); all O(batch) compute — the embed feature construction, the
full dense matmul and the bias add — runs on device.

Device kernel (per core):
  - xt [2048, 1024] (x shard, feature-major) streamed in as [128,128] k-tiles
  - wt [2176, 2048] (W^T) resident in SBUF
  - 128 embed features built on device: indirect-DMA row gather from xt
    + per-partition scale; forms contraction k-tile 16
  - out[m*128:(m+1)*128, :] = sum_k xt_tile[k,m].T @ wt_tile[k] + bias
    (PE matmul in float32r, PSUM fp32 accumulation over 17 k-tiles)
"""

import numpy as np
from contextlib import ExitStack

OUT = 2048
IN_BASE = 2048
N_EMBED = 64
IN_TOT = IN_BASE + 2 * N_EMBED  # 2176
BATCH = 8192
N_CORES = 8
B_CORE = BATCH // N_CORES       # 1024
P = 128
K_TILES = IN_TOT // P           # 17
M_TILES = B_CORE // P           # 8
N_SPLIT = 4                     # 2048 out cols in 4 x 512 (one PSUM bank each)

_CACHED = {}


def _build_nc():
    import concourse.bass as bass
    import concourse.mybir as mybir
    import concourse.tile as tile
    from concourse import bacc
    from concourse.tile_rust import add_dep_helper

    f32 = mybir.dt.float32
    f32r = mybir.dt.float32r
    i32 = mybir.dt.int32

    nc = bacc.Bacc("TRN2", target_bir_lowering=False, debug=False,
                   num_devices=N_CORES)

    xt = nc.dram_tensor("xt", [B_CORE, (K_TILES - 1) * P], f32r,
                        kind="ExternalInput")  # pre-tiled: row m*128+p
    xg = nc.dram_tensor("xg", [IN_BASE, B_CORE], f32r, kind="ExternalInput")
    wt = nc.dram_tensor("wt", [IN_TOT, OUT], f32r, kind="ExternalInput")
    bias = nc.dram_tensor("bias", [P, OUT], f32, kind="ExternalInput")
    emb_q = nc.dram_tensor("emb_q", [P, 1], i32, kind="ExternalInput")
    emb_a = nc.dram_tensor("emb_a", [P, 1], f32, kind="ExternalInput")
    out = nc.dram_tensor("out", [B_CORE, OUT], f32, kind="ExternalOutput")

    NW = 512  # wt stream chunk width == one fp32 PSUM bank

    with tile.TileContext(nc) as tc:
        with ExitStack() as ctx:
            wt_pool = ctx.enter_context(tc.tile_pool(name="wt", bufs=18))
            xt_pool = ctx.enter_context(tc.tile_pool(name="xt", bufs=M_TILES))
            small_pool = ctx.enter_context(tc.tile_pool(name="small", bufs=1))
            out_pool = ctx.enter_context(tc.tile_pool(name="out", bufs=4))
            psum_pool = ctx.enter_context(
                tc.tile_pool(name="psum", bufs=8, space="PSUM"))

            # gpsimd SWDGE queue order matters (FIFO): embed params + first
            # xt tile + gather first, remaining xt tiles, bias last (only
            # needed at first evac). Both HWDGE queues stream wt chunks.
            q_t = small_pool.tile([P, 1], i32, tag="q")
            nc.sync.dma_start(out=q_t[:], in_=emb_q.ap())
            a_t = small_pool.tile([P, 1], f32, tag="a")
            nc.sync.dma_start(out=a_t[:], in_=emb_a.ap())

            xt_tiles = []

            # ring assignment for the startup-critical path: the first MM
            # needs xt0 AND wt chunk 0. chunk 0 goes at the HEAD of the sync
            # ring; xt0 alone at the head of the scalar ring (its chunks
            # queue behind 1.1MB only); xt1-7 stream on gpsimd at ~1 tile
            # per 3us, just ahead of PE's 3.9us-per-m-block consumption.
            xt_dmas = []

            def load_xt(m):
                xt_m = xt_pool.tile([P, (K_TILES - 1) * P], f32r, tag="xt",
                                    name=f"xt_m{m}")
                eng = nc.scalar if m == 0 else nc.gpsimd
                di = eng.dma_start(
                    out=xt_m[:], in_=xt.ap()[m * P:(m + 1) * P, :])
                xt_dmas.append(di)
                xt_tiles.append(xt_m)

            for m in range(4):
                load_xt(m)

            # embed features: gather parent rows of x (feature-major copy),
            # scale by alpha. partition j = expanded feature 2048+j; forms
            # k-tile 16 of x_exp^T. Emitted mid-xt-stream: needed by the end
            # of round 0a; its Q7-side sem wait (on q_t) is satisfied by
            # then, so it does not stall the SWDGE ring.
            emb_raw = small_pool.tile([P, B_CORE], f32r, tag="emb_raw")
            nc.gpsimd.indirect_dma_start(
                out=emb_raw[:],
                out_offset=None,
                in_=xg.ap(),
                in_offset=bass.IndirectOffsetOnAxis(ap=q_t[:, 0:1], axis=0),
            )
            emb_t = small_pool.tile([P, B_CORE], f32r, tag="emb")
            nc.vector.tensor_scalar_mul(
                emb_t[:], emb_raw[:].bitcast(f32), a_t[:, 0:1])

            for m in range(4, M_TILES):
                load_xt(m)

            bias_t = small_pool.tile([P, OUT], f32, tag="bias")
            nc.gpsimd.dma_start(out=bias_t[:], in_=bias.ap())

            # stream W^T n-major in [128, CK*512] chunks alternating across
            # both HWDGE queues; 8 single-bank PSUM accumulators = all 8
            # m-tiles in flight per n, so PE starts as soon as chunk 0 lands.
            CK = 2
            wt_ap3 = wt.ap().rearrange("(k p) n -> p k n", p=P)  # [128,17,2048]
            k_chunks = [(k0, min(CK, K_TILES - k0))
                        for k0 in range(0, K_TILES, CK)]
            dma_engines = [nc.sync, nc.scalar]
            ci = 0

            def load_wt_chunk(n, k0, klen):
                wck = wt_pool.tile([P, CK * NW], f32r, tag="wck",
                                   name=f"wck_n{n}_k{k0}")
                nc_dma = dma_engines[load_wt_chunk.ci % 2]
                load_wt_chunk.ci += 1
                di = nc_dma.dma_start(
                    out=wck[:, :klen * NW].rearrange(
                        "p (k c) -> p k c", k=klen),
                    in_=wt_ap3[:, k0:k0 + klen, n * NW:(n + 1) * NW])
                if n == 1:
                    # round 1 yields the t0 window to round 0's loads, but
                    # only until xt3 lands (gating on the last xt tile can
                    # block round-0 chunks via scheduler reordering).
                    add_dep_helper(di.ins, xt_dmas[3].ins, sync=True,
                                   reason="wt r1 prefetch yields to xt0-3")
                elif n > 1:
                    # rounds 2+: yield startup bandwidth to the xt stream
                    add_dep_helper(di.ins, xt_dmas[-1].ins, sync=True,
                                   reason="wt prefetch yields to xt stream")
                return wck

            load_wt_chunk.ci = 0

            def mm(psum, k, m, wck, kk):
                if k < K_TILES - 1:
                    lhsT = xt_tiles[m][:, k * P:(k + 1) * P]
                else:
                    lhsT = emb_t[:, m * P:(m + 1) * P]
                nc.tensor.matmul(
                    psum[:],
                    lhsT=lhsT,
                    rhs=wck[:, kk * NW:(kk + 1) * NW],
                    start=(k == 0),
                    stop=(k == K_TILES - 1),
                )

            for n in range(N_SPLIT):
                psums = [psum_pool.tile([P, NW], f32, tag="ps",
                                        name=f"ps_n{n}_m{m}")
                         for m in range(M_TILES)]
                if n == 0:
                    # round 0: two half-rounds (m0-3 then m4-7), k-inner
                    # within each — a half-round needs only 4 xt tiles and
                    # consumes each wt chunk at 4 MMs/chunk, matching the
                    # chunk arrival rate; the chunks are reused by the
                    # second half from SBUF.
                    wcks = [load_wt_chunk(n, k0, klen)
                            for k0, klen in k_chunks]
                    for mg in (range(0, 4), range(4, M_TILES)):
                        for (k0, klen), wck in zip(k_chunks, wcks):
                            for kk in range(klen):
                                for m in mg:
                                    mm(psums[m], k0 + kk, m, wck, kk)
                else:
                    # rounds 1-3: chunks are fully prefetched by round
                    # start, so iterate m-outer — psum completions stagger
                    # through the round, spreading evac+store (and freeing
                    # PSUM banks for the next round progressively) instead
                    # of serializing at round end.
                    wcks = [load_wt_chunk(n, k0, klen)
                            for k0, klen in k_chunks]
                    for m in range(M_TILES):
                        for (k0, klen), wck in zip(k_chunks, wcks):
                            for kk in range(klen):
                                mm(psums[m], k0 + kk, m, wck, kk)
                for m in range(M_TILES):
                    ot = out_pool.tile([P, NW], f32, tag="ot")
                    nc.vector.tensor_add(
                        ot[:], psums[m][:], bias_t[:, n * NW:(n + 1) * NW])
                    # last round: HWDGE rings are idle once the wt stream
                    # ends — use them for the final stores (faster receipts)
                    st_eng = (dma_engines[m % 2] if n == N_SPLIT - 1
                              else nc.gpsimd)
                    st_eng.dma_start(
                        out=out.ap()[m * P:(m + 1) * P, n * NW:(n + 1) * NW],
                        in_=ot[:])

    nc.compile()
    return nc


def _host_prep(inputs):
    x = np.ascontiguousarray(np.asarray(inputs["x"], dtype=np.float32))
    wv = np.asarray(inputs["weight_vals"], dtype=np.float32)
    wr = np.asarray(inputs["weight_rows"]).astype(np.int64)
    wc = np.asarray(inputs["weight_cols"]).astype(np.int64)
    bv = np.asarray(inputs["bias_vals"], dtype=np.float32)
    bi = np.asarray(inputs["bias_idx"]).astype(np.int64)
    e0v = np.asarray(inputs["embed0_vals"], dtype=np.float32)
    e0p = np.asarray(inputs["embed0_parents"]).astype(np.int64)
    e1v = np.asarray(inputs["embed1_vals"], dtype=np.float32)
    e1p = np.asarray(inputs["embed1_parents"]).astype(np.int64)

    # dense W^T [IN_TOT, OUT] (coalesce: duplicates sum)
    wt = np.bincount(wc * OUT + wr, weights=wv,
                     minlength=IN_TOT * OUT).reshape(IN_TOT, OUT)
    wt = np.ascontiguousarray(wt.astype(np.float32))

    b = np.bincount(bi, weights=bv, minlength=OUT).astype(np.float32)
    bias_bcast = np.ascontiguousarray(
        np.broadcast_to(b[None, :], (P, OUT)).astype(np.float32))

    # resolve embed parent chains to direct (row-in-x, multiplier) pairs
    q = np.empty(2 * N_EMBED, dtype=np.int32)
    a = np.empty(2 * N_EMBED, dtype=np.float32)
    q[:N_EMBED] = e0p
    a[:N_EMBED] = e0v
    for j in range(N_EMBED):
        p = int(e1p[j])
        if p < IN_BASE:
            q[N_EMBED + j] = p
            a[N_EMBED + j] = e1v[j]
        else:
            t = p - IN_BASE
            q[N_EMBED + j] = e0p[t]
            a[N_EMBED + j] = e1v[j] * e0v[t]

    xts = []
    xgs = []
    for i in range(N_CORES):
        xs = x[i * B_CORE:(i + 1) * B_CORE]
        # SBUF-tiled layout: row m*128+p, col k*128+f  ==  xs[m*128+f, k*128+p]
        xts.append(np.ascontiguousarray(
            xs.reshape(M_TILES, P, K_TILES - 1, P)
              .transpose(0, 3, 2, 1).reshape(B_CORE, (K_TILES - 1) * P)))
        xgs.append(np.ascontiguousarray(xs.T))
    return xts, xgs, wt, bias_bcast, q.reshape(P, 1), a.reshape(P, 1)


def kernel(**inputs) -> np.ndarray:
    import time
    from concourse.bass_utils import run_bass_kernel_spmd

    if "nc" not in _CACHED:
        _CACHED["nc"] = _build_nc()
    nc = _CACHED["nc"]

    xts, xgs, wt, bias_bcast, q, a = _host_prep(inputs)
    in_maps = [
        dict(xt=xts[i], xg=xgs[i], wt=wt, bias=bias_bcast, emb_q=q, emb_a=a)
        for i in range(N_CORES)
    ]
    res = None
    last_exc = None
    for attempt in range(3):
        try:
            res = run_bass_kernel_spmd(nc, in_maps,
                                       core_ids=list(range(N_CORES)))
            break
        except Exception as e:  # transient device/runtime hiccups
            last_exc = e
            time.sleep(2.0)
    if res is None:
        raise last_exc
    out = np.concatenate([res.results[i]["out"] for i in range(N_CORES)],
                         axis=0)
    return np.ascontiguousarray(out.astype(np.float32))



# revision 2
# speedup vs baseline: 1.1156x; 1.1156x over previous
"""Trainium2 Bass kernel for nn_ExpandingLinear.

Computation (see reference):
    x_exp = concat([x, x[:, p0] * v0, x_exp1[:, p1] * v1], axis=1)   # [B, 2176]
    W     = scatter_add(weight_vals at [weight_rows, weight_cols])    # [2048, 2176]
    b     = scatter_add(bias_vals at bias_idx)                        # [2048]
    out   = x_exp @ W.T + b                                           # [B, 2048]

Key transform: every expanded feature is x[:, q_j] * a_j for a base column
q_j (embed chains resolved on host), so its weight column can be folded into
the base weight matrix: W'[:, q_j] += a_j * W[:, 2048+j]. The device then
computes a plain dense out = x @ W'^T + b with K = 2048 (16 k-tiles of 128).

Sharding: data-parallel over the batch dim across 8 NeuronCores (1024 rows
per core); W'/bias replicated. Host prep is sharding + parameter/layout work
(batch split, transpose, COO densification, fold, fp16 conversion); the
full dense matmul and bias add run on device.

Device kernel (per core), all fp16 operands, fp32 PSUM accumulation:
  - xk[k]   [128, 1024]  x^T k-slice (feature-part, batch-free), 16 tiles
  - wck[k2,n] [128, 1024] W'^T chunk (2 k-tiles x 512 out-cols), 32 tiles
  - out[m*128:.., n*512:..] = sum_k xk[k][:,m].T @ wck + bias  (4 n-rounds,
    8 m-tiles, 8 single-bank PSUM accumulators)
  - round 0 is k-outer/m-inner so the first matmul needs only xk[0]+wck[0,0];
    rounds 1-3 are m-outer/k-inner (weights already SBUF-resident) so psum
    evacuations stagger through the round.
  - a short warm-up matmul train runs during the DMA dead time so the PE HAM
    clock gate is at 2.4 GHz when real work starts.
  - outputs are stored fp16 (host upcasts); max rel err ~5e-4 vs fp32 ref.
"""

import numpy as np
from contextlib import ExitStack

OUT = 2048
IN_BASE = 2048
N_EMBED = 64
IN_TOT = IN_BASE + 2 * N_EMBED  # 2176
BATCH = 8192
N_CORES = 8
B_CORE = BATCH // N_CORES       # 1024
P = 128
K_TILES = IN_BASE // P          # 16 (embed columns folded into base W)
M_TILES = B_CORE // P           # 8
N_SPLIT = 4                     # 2048 out cols in 4 x 512 (one PSUM bank each)
NW = 512
K2 = K_TILES // 2               # wt chunk granularity: 2 k-tiles
N_WU = 20                       # PE clock warm-up matmuls

_CACHED = {}


def _build_nc():
    import concourse.bass as bass
    import concourse.mybir as mybir
    import concourse.tile as tile
    from concourse import bacc

    f32 = mybir.dt.float32
    f16 = mybir.dt.float16

    nc = bacc.Bacc("TRN2", target_bir_lowering=False, debug=False,
                   num_devices=N_CORES)

    xk = nc.dram_tensor("xk", [K_TILES, P, B_CORE], f16, kind="ExternalInput")
    wt = nc.dram_tensor("wt", [K2, N_SPLIT, P, 2 * NW], f16,
                        kind="ExternalInput")
    bias = nc.dram_tensor("bias", [P, OUT], f32, kind="ExternalInput")
    out = nc.dram_tensor("out", [N_SPLIT, M_TILES, P, NW], f16,
                         kind="ExternalOutput")

    with tile.TileContext(nc) as tc:
        with ExitStack() as ctx:
            xk_pool = ctx.enter_context(tc.tile_pool(name="xk", bufs=K_TILES))
            wt_pool = ctx.enter_context(tc.tile_pool(name="wt", bufs=32))
            small_pool = ctx.enter_context(tc.tile_pool(name="small", bufs=1))
            out_pool = ctx.enter_context(tc.tile_pool(name="out", bufs=8))
            psum_pool = ctx.enter_context(
                tc.tile_pool(name="psum", bufs=8, space="PSUM"))

            # PE warm-up: releases the HAM clock throttle (4096-cycle
            # activity window) during the DMA-startup dead time so the real
            # matmul stream starts at 2.4 GHz instead of 1.2 GHz.
            wu = small_pool.tile([P, P + NW], f16, tag="wu")
            nc.vector.memset(wu[:], 1.0)
            wu_ps = psum_pool.tile([P, NW], f32, tag="ps", name="wu")
            for _ in range(N_WU):
                nc.tensor.matmul(wu_ps[:], lhsT=wu[:, :P], rhs=wu[:, P:],
                                 start=True, stop=True)

            # ---- DMA streams (per-queue FIFO order == consumption order)
            # sync:   xk even slices, wt n1, wt n3 (k2 0-3)
            # scalar: wck(0,0) first (first matmul needs it), xk odd,
            #         wt n2, wt n3 (k2 4-7)
            # gpsimd: wck(1..7, 0) for round 0, bias, then round 0-2 stores
            xk_tiles = [None] * K_TILES
            wck_tiles = [[None] * N_SPLIT for _ in range(K2)]

            def load_xk(k, eng):
                t = xk_pool.tile([P, B_CORE], f16, tag="xk", name=f"xk{k}")
                eng.dma_start(out=t[:], in_=xk.ap()[k])
                xk_tiles[k] = t

            def load_wck(k2i, n, eng):
                t = wt_pool.tile([P, 2 * NW], f16, tag="wck",
                                 name=f"wck_k{k2i}_n{n}")
                eng.dma_start(out=t[:], in_=wt.ap()[k2i, n])
                wck_tiles[k2i][n] = t

            load_xk(0, nc.sync)
            load_wck(0, 0, nc.scalar)
            for k in range(2, K_TILES, 2):
                load_xk(k, nc.sync)
            for k in range(1, K_TILES, 2):
                load_xk(k, nc.scalar)
            for k2i in range(1, K2):
                load_wck(k2i, 0, nc.gpsimd)
            for k2i in range(K2):
                load_wck(k2i, 1, nc.sync)
            for k2i in range(K2):
                load_wck(k2i, 2, nc.scalar)
            for k2i in range(K2 // 2):
                load_wck(k2i, 3, nc.sync)
            for k2i in range(K2 // 2, K2):
                load_wck(k2i, 3, nc.scalar)

            bias_t = small_pool.tile([P, OUT], f32, tag="bias")
            nc.gpsimd.dma_start(out=bias_t[:], in_=bias.ap())

            def mm(psum, k, m, n):
                nc.tensor.matmul(
                    psum[:],
                    lhsT=xk_tiles[k][:, m * P:(m + 1) * P],
                    rhs=wck_tiles[k // 2][n][:, (k % 2) * NW:(k % 2 + 1) * NW],
                    start=(k == 0),
                    stop=(k == K_TILES - 1),
                )

            def evac(psum, m, n):
                ot = out_pool.tile([P, NW], f16, tag="ot")
                nc.vector.tensor_add(
                    ot[:], psum[:], bias_t[:, n * NW:(n + 1) * NW])
                st_eng = ([nc.sync, nc.scalar][m % 2] if n == N_SPLIT - 1
                          else nc.gpsimd)
                st_eng.dma_start(out=out.ap()[n, m], in_=ot[:])

            for n in range(N_SPLIT):
                psums = [psum_pool.tile([P, NW], f32, tag="ps",
                                        name=f"ps_n{n}_m{m}")
                         for m in range(M_TILES)]
                if n == 0:
                    # k-outer: matmul 1 needs only xk[0] + wck[0,0]; each
                    # k-step's data (256+128 KB) arrives well within the
                    # 8 x 213ns it takes the PE to consume the previous one.
                    for k in range(K_TILES):
                        for m in range(M_TILES):
                            mm(psums[m], k, m, n)
                else:
                    # weights all SBUF-resident: m-outer staggers the psum
                    # completions so evac+store spread through the round.
                    for m in range(M_TILES):
                        for k in range(K_TILES):
                            mm(psums[m], k, m, n)
                for m in range(M_TILES):
                    evac(psums[m], m, n)

    nc.compile()
    return nc


def _host_prep(inputs):
    x = np.asarray(inputs["x"], dtype=np.float32)
    wv = np.asarray(inputs["weight_vals"], dtype=np.float64)
    wr = np.asarray(inputs["weight_rows"]).astype(np.int64)
    wc = np.asarray(inputs["weight_cols"]).astype(np.int64)
    bv = np.asarray(inputs["bias_vals"], dtype=np.float64)
    bi = np.asarray(inputs["bias_idx"]).astype(np.int64)
    e0v = np.asarray(inputs["embed0_vals"], dtype=np.float64)
    e0p = np.asarray(inputs["embed0_parents"]).astype(np.int64)
    e1v = np.asarray(inputs["embed1_vals"], dtype=np.float64)
    e1p = np.asarray(inputs["embed1_parents"]).astype(np.int64)

    # dense W [OUT, IN_TOT] (coalesce: duplicates sum)
    W = np.bincount(wr * IN_TOT + wc, weights=wv,
                    minlength=OUT * IN_TOT).reshape(OUT, IN_TOT)

    # resolve embed parent chains to (base column, multiplier)
    q = np.empty(2 * N_EMBED, dtype=np.int64)
    a = np.empty(2 * N_EMBED, dtype=np.float64)
    q[:N_EMBED] = e0p
    a[:N_EMBED] = e0v
    for j in range(N_EMBED):
        p = int(e1p[j])
        if p < IN_BASE:
            q[N_EMBED + j] = p
            a[N_EMBED + j] = e1v[j]
        else:
            q[N_EMBED + j] = e0p[p - IN_BASE]
            a[N_EMBED + j] = e1v[j] * e0v[p - IN_BASE]

    # fold embed weight columns into the base weight matrix
    Wf = W[:, :IN_BASE].copy()
    np.add.at(Wf.T, q, (a[None, :] * W[:, IN_BASE:]).T)

    # wt[k2, n, p, kk*512+j] = Wf.T[(2*k2+kk)*128+p, n*512+j]
    wt = np.ascontiguousarray(
        Wf.T.reshape(K2, 2, P, N_SPLIT, NW)
          .transpose(0, 3, 2, 1, 4).reshape(K2, N_SPLIT, P, 2 * NW)
          .astype(np.float16))

    b = np.bincount(bi, weights=bv, minlength=OUT).astype(np.float32)
    bias_bcast = np.ascontiguousarray(
        np.broadcast_to(b[None, :], (P, OUT)).astype(np.float32))

    xks = []
    for i in range(N_CORES):
        xs = x[i * B_CORE:(i + 1) * B_CORE]
        # xk[k, p, :] = xs.T[k*128+p, :]
        xks.append(np.ascontiguousarray(
            xs.T.astype(np.float16).reshape(K_TILES, P, B_CORE)))
    return xks, wt, bias_bcast


def kernel(**inputs) -> np.ndarray:
    import time
    from concourse.bass_utils import run_bass_kernel_spmd

    if "nc" not in _CACHED:
        _CACHED["nc"] = _build_nc()
    nc = _CACHED["nc"]

    xks, wt, bias_bcast = _host_prep(inputs)
    in_maps = [dict(xk=xks[i], wt=wt, bias=bias_bcast)
               for i in range(N_CORES)]
    res = None
    last_exc = None
    for attempt in range(3):
        try:
            res = run_bass_kernel_spmd(nc, in_maps,
                                       core_ids=list(range(N_CORES)))
            break
        except Exception as e:  # transient device/runtime hiccups
            last_exc = e
            time.sleep(2.0)
    if res is None:
        raise last_exc
    parts = []
    for i in range(N_CORES):
        o4 = res.results[i]["out"]  # [4, 8, 128, 512] fp16
        parts.append(o4.transpose(1, 2, 0, 3).reshape(B_CORE, OUT))
    return np.concatenate(parts, axis=0).astype(np.float32)


# revision 9
# speedup vs baseline: 1.2802x; 1.1475x over previous
"""Trainium2 Bass kernel for nn_ExpandingLinear.

Computation (see reference):
    x_exp = concat([x, x[:, p0] * v0, x_exp1[:, p1] * v1], axis=1)   # [B, 2176]
    W     = scatter_add(weight_vals at [weight_rows, weight_cols])    # [2048, 2176]
    b     = scatter_add(bias_vals at bias_idx)                        # [2048]
    out   = x_exp @ W.T + b                                           # [B, 2048]

Key transform: every expanded feature is x[:, q_j] * a_j for a base column
q_j (embed chains resolved on host), so its weight column can be folded into
the base weight matrix: W'[:, q_j] += a_j * W[:, 2048+j]. The device then
computes a plain dense out = x @ W'^T + b with K = 2048 (16 k-tiles of 128).

Sharding: data-parallel over the batch dim across 8 NeuronCores (1024 rows
per core); W'/bias replicated. Host prep is sharding + parameter/layout work
(batch split, transpose, COO densification, fold, fp16 conversion); the
full dense matmul and bias add run on device.

Device kernel (per core), all bf16 operands, fp32 PSUM accumulation:
  - xk[k]   [128, 1024]  x^T k-slice (feature-part, batch-free), 16 tiles
  - wck[k2,n] [128, 1024] W'^T chunk (2 k-tiles x 512 out-cols), 32 tiles
  - out[m*128:.., n*512:..] = sum_k xk[k][:,m].T @ wck + bias  (8 m-tiles,
    4 n-blocks of 512, 8 single-bank PSUM accumulators)
  - round 0 (n=0) is k-outer/m-inner so the first matmul needs only
    xk[0]+wck[0,0]; rounds 1-3 run m-outer/k-outer/n-inner (weights already
    SBUF-resident) so psum evacuations stagger and 3 consecutive matmuls
    share one stationary load.
  - outputs are stored bf16 (host upcasts); max rel err ~2e-3 vs fp32 ref.
"""

import numpy as np
from contextlib import ExitStack

OUT = 2048
IN_BASE = 2048
N_EMBED = 64
IN_TOT = IN_BASE + 2 * N_EMBED  # 2176
BATCH = 8192
N_CORES = 8
B_CORE = BATCH // N_CORES       # 1024
P = 128
K_TILES = IN_BASE // P          # 16 (embed columns folded into base W)
M_TILES = B_CORE // P           # 8
N_SPLIT = 4                     # 2048 out cols in 4 x 512 (one PSUM bank each)
NW = 512
K2 = K_TILES // 2               # wt chunk granularity: 2 k-tiles

_CACHED = {}


def _build_nc():
    import concourse.bass as bass
    import concourse.mybir as mybir
    import concourse.tile as tile
    from concourse import bacc

    f32 = mybir.dt.float32
    f16 = mybir.dt.bfloat16

    nc = bacc.Bacc("TRN2", target_bir_lowering=False, debug=False,
                   num_devices=N_CORES)

    xk = nc.dram_tensor("xk", [K_TILES, P, B_CORE], f16, kind="ExternalInput")
    wt = nc.dram_tensor("wt", [K2, N_SPLIT, P, 2 * NW], f16,
                        kind="ExternalInput")
    bias = nc.dram_tensor("bias", [P, OUT], f32, kind="ExternalInput")
    out = nc.dram_tensor("out", [N_SPLIT, M_TILES, P, NW], f16,
                         kind="ExternalOutput")

    with tile.TileContext(nc) as tc:
        with ExitStack() as ctx:
            xk_pool = ctx.enter_context(tc.tile_pool(name="xk", bufs=K_TILES))
            wt_pool = ctx.enter_context(tc.tile_pool(name="wt", bufs=32))
            small_pool = ctx.enter_context(tc.tile_pool(name="small", bufs=1))
            out_pool = ctx.enter_context(tc.tile_pool(name="out", bufs=8))
            psum_pool = ctx.enter_context(
                tc.tile_pool(name="psum", bufs=8, space="PSUM"))

            # ---- DMA streams (per-queue FIFO order == consumption order)
            # sync:   xk even slices, wt n1, wt n3 (k2 0-3)
            # scalar: wck(0,0) first (first matmul needs it), xk odd,
            #         wt n2, wt n3 (k2 4-7)
            # gpsimd: wck(1..7, 0) for round 0, bias, then round 0-2 stores
            xk_tiles = [None] * K_TILES
            wck_tiles = [[None] * N_SPLIT for _ in range(K2)]

            def load_xk(k, eng):
                t = xk_pool.tile([P, B_CORE], f16, tag="xk", name=f"xk{k}")
                eng.dma_start(out=t[:], in_=xk.ap()[k])
                xk_tiles[k] = t

            def load_wck(k2i, n, eng):
                t = wt_pool.tile([P, 2 * NW], f16, tag="wck",
                                 name=f"wck_k{k2i}_n{n}")
                eng.dma_start(out=t[:], in_=wt.ap()[k2i, n])
                wck_tiles[k2i][n] = t

            load_xk(0, nc.sync)
            load_wck(0, 0, nc.scalar)
            for k in range(2, K_TILES, 2):
                load_xk(k, nc.sync)
            for k in range(1, K_TILES, 2):
                load_xk(k, nc.scalar)
            for k2i in range(1, K2):
                load_wck(k2i, 0, nc.gpsimd)
            for k2i in range(K2):
                load_wck(k2i, 1, nc.sync)
            for k2i in range(K2):
                load_wck(k2i, 2, nc.scalar)
            for k2i in range(K2 // 2):
                load_wck(k2i, 3, nc.sync)
            for k2i in range(K2 // 2, K2):
                load_wck(k2i, 3, nc.scalar)

            bias_t = small_pool.tile([P, OUT], f32, tag="bias")
            nc.gpsimd.dma_start(out=bias_t[:], in_=bias.ap())

            def mm(psum, k, m, n):
                nc.tensor.matmul(
                    psum[:],
                    lhsT=xk_tiles[k][:, m * P:(m + 1) * P],
                    rhs=wck_tiles[k // 2][n][:, (k % 2) * NW:(k % 2 + 1) * NW],
                    start=(k == 0),
                    stop=(k == K_TILES - 1),
                )

            def evac(psum, m, n):
                ot = out_pool.tile([P, NW], f16, tag="ot")
                nc.vector.tensor_add(
                    ot[:], psum[:], bias_t[:, n * NW:(n + 1) * NW])
                st_eng = ([nc.sync, nc.scalar][m % 2] if n == N_SPLIT - 1
                          else nc.gpsimd)
                st_eng.dma_start(out=out.ap()[n, m], in_=ot[:])

            # round 0 (n=0), k-outer: matmul 1 needs only xk[0] + wck[0,0];
            # each k-step's data (256+128 KB) arrives well within the
            # 8 x 213ns it takes the PE to consume the previous one.
            psums0 = [psum_pool.tile([P, NW], f32, tag="ps",
                                     name=f"ps_n0_m{m}")
                      for m in range(M_TILES)]
            for k in range(K_TILES):
                for m in range(M_TILES):
                    mm(psums0[m], k, m, 0)
            for m in range(M_TILES):
                evac(psums0[m], m, 0)

            # rounds 1-3 merged, weights all SBUF-resident: m-outer staggers
            # psum completions so evac+store spread out; n-innermost lets 3
            # consecutive matmuls share the same stationary operand (one
            # LDWEIGHTS per (m, k)).
            for m in range(M_TILES):
                psums = [psum_pool.tile([P, NW], f32, tag="ps",
                                        name=f"ps_m{m}_n{n}")
                         for n in range(1, N_SPLIT)]
                for k in range(K_TILES):
                    for n in range(1, N_SPLIT):
                        mm(psums[n - 1], k, m, n)
                for n in range(1, N_SPLIT):
                    evac(psums[n - 1], m, n)

    nc.compile()
    return nc


def _host_prep(inputs):
    x = np.asarray(inputs["x"], dtype=np.float32)
    wv = np.asarray(inputs["weight_vals"], dtype=np.float64)
    wr = np.asarray(inputs["weight_rows"]).astype(np.int64)
    wc = np.asarray(inputs["weight_cols"]).astype(np.int64)
    bv = np.asarray(inputs["bias_vals"], dtype=np.float64)
    bi = np.asarray(inputs["bias_idx"]).astype(np.int64)
    e0v = np.asarray(inputs["embed0_vals"], dtype=np.float64)
    e0p = np.asarray(inputs["embed0_parents"]).astype(np.int64)
    e1v = np.asarray(inputs["embed1_vals"], dtype=np.float64)
    e1p = np.asarray(inputs["embed1_parents"]).astype(np.int64)

    # dense W [OUT, IN_TOT] (coalesce: duplicates sum)
    W = np.bincount(wr * IN_TOT + wc, weights=wv,
                    minlength=OUT * IN_TOT).reshape(OUT, IN_TOT)

    # resolve embed parent chains to (base column, multiplier)
    q = np.empty(2 * N_EMBED, dtype=np.int64)
    a = np.empty(2 * N_EMBED, dtype=np.float64)
    q[:N_EMBED] = e0p
    a[:N_EMBED] = e0v
    for j in range(N_EMBED):
        p = int(e1p[j])
        if p < IN_BASE:
            q[N_EMBED + j] = p
            a[N_EMBED + j] = e1v[j]
        else:
            q[N_EMBED + j] = e0p[p - IN_BASE]
            a[N_EMBED + j] = e1v[j] * e0v[p - IN_BASE]

    # fold embed weight columns into the base weight matrix
    Wf = W[:, :IN_BASE].copy()
    np.add.at(Wf.T, q, (a[None, :] * W[:, IN_BASE:]).T)

    import ml_dtypes
    # wt[k2, n, p, kk*512+j] = Wf.T[(2*k2+kk)*128+p, n*512+j]
    wt = np.ascontiguousarray(
        Wf.T.reshape(K2, 2, P, N_SPLIT, NW)
          .transpose(0, 3, 2, 1, 4).reshape(K2, N_SPLIT, P, 2 * NW)
          .astype(np.float32).astype(ml_dtypes.bfloat16))

    b = np.bincount(bi, weights=bv, minlength=OUT).astype(np.float32)
    bias_bcast = np.ascontiguousarray(
        np.broadcast_to(b[None, :], (P, OUT)).astype(np.float32))

    xks = []
    for i in range(N_CORES):
        xs = x[i * B_CORE:(i + 1) * B_CORE]
        # xk[k, p, :] = xs.T[k*128+p, :]
        xks.append(np.ascontiguousarray(
            xs.T.astype(ml_dtypes.bfloat16).reshape(K_TILES, P, B_CORE)))
    return xks, wt, bias_bcast


def kernel(**inputs) -> np.ndarray:
    import time
    from concourse.bass_utils import run_bass_kernel_spmd

    if "nc" not in _CACHED:
        _CACHED["nc"] = _build_nc()
    nc = _CACHED["nc"]

    xks, wt, bias_bcast = _host_prep(inputs)
    in_maps = [dict(xk=xks[i], wt=wt, bias=bias_bcast)
               for i in range(N_CORES)]
    res = None
    last_exc = None
    for attempt in range(3):
        try:
            res = run_bass_kernel_spmd(nc, in_maps,
                                       core_ids=list(range(N_CORES)))
            break
        except Exception as e:  # transient device/runtime hiccups
            last_exc = e
            time.sleep(2.0)
    if res is None:
        raise last_exc
    parts = []
    for i in range(N_CORES):
        o4 = res.results[i]["out"]  # [4, 8, 128, 512] fp16
        parts.append(o4.transpose(1, 2, 0, 3).reshape(B_CORE, OUT))
    return np.concatenate(parts, axis=0).astype(np.float32)
